# revision 7
# baseline (speedup 1.0000x reference)
"""KoLeo loss kernel for 8 Trainium2 NeuronCores — symmetric (half-matrix)
variant.

Reference computation (B=16384, D=1024):
    xn  = x / max(||x||_2, 1e-12)          # row L2-normalize
    sim = xn @ xn.T                        # B x B cosine similarity
    max_sim[i] = max_{j != i} sim[i, j]    # nearest neighbor (excl. self)
    out = -mean(log(sqrt(2 - 2*max_sim + 1e-8)))

sim is symmetric, so only the upper triangle of 512x512 super-blocks is
computed (~53% of the matmul work of the dense baseline). Each computed
super-block (I, J), I < J, serves rows of I via its row-max and rows of J
via its column-max:
  - row-max: DVE free-axis reduce straight from PSUM.
  - column-max: ACT copies the 4 PSUM chunks to SBUF (Pool cannot touch
    PSUM on TRN2), Pool runs a partition_all_reduce (max over the 128
    partitions for each of the 4x512 free elements), and one replicated
    partition row [1, 4*512] is DMA'd out per super-block. The host takes
    the max over the 4 chunk rows and scatters into the global row maxima.
This keeps the Tensor engine the critical path: per core 66 super-blocks
= 1056 fp8 DoubleRow matmuls (~229 us at the 157 TF/s fp8 roofline, vs
437 us for the dense baseline), with DVE ~156 us, ACT ~150 us and Pool
~180 us hidden underneath.

Work distribution ("pencil window", SPMD-uniform): global super-row G is
owned by core c = G % 8. Each core holds its 4 owned super-rows resident
(local positions 8a after a per-core rotation of x.T columns by 512*c) and
computes super-blocks (I, I+w mod 32) for w = 0..16 (a in {0,1}) or
w = 0..15 (a in {2,3}). Every unordered pair of super-blocks is covered
exactly once across the fleet (528 total); w=0 is the diagonal block,
where a -2*scale^2*I constant is added at the self-similarity positions
before the row max so the self-match never wins.

Host: pre-normalizes rows (f64), scales by 8 and casts to fp8e4m3,
pre-rotates columns per core; post-merges row/column maxima across cores
and applies the scalar log epilogue in f64.
"""

import sys

if "/opt/trn_rl_repo" not in sys.path:
    sys.path.insert(0, "/opt/trn_rl_repo")

import numpy as np
import ml_dtypes

import concourse.bass as bass  # noqa: F401  (import keeps bass registered)
import concourse.bass_isa as bass_isa
import concourse.mybir as mybir
import concourse.tile as tile
from concourse import bacc
from concourse.bass_utils import run_bass_kernel_spmd

P = 128          # SBUF partitions
NBLK = 512       # super-block side (= one PSUM bank of f32 per 128 rows)
EPS = 1e-8

B = 16384        # rows of x
D = 1024         # feature dim
N_CORES = 8
NSB = B // NBLK  # 32 super-blocks per matrix side
KCH = D // P     # 8 contraction chunks of 128
KSTEP = 2        # fp8 DoubleRow: K chunks of 256 per matmul
NA = 4           # owned super-rows per core (global stride 8)
FP8_SCALE = 8.0


def _windows():
    """Program-order (a, w) list. w=0 is the diagonal super-block.

    a in {0,1} get w up to 16, a in {2,3} up to 15: the distance-16 pairs
    {i, i+16} are covered once by the a0/a1 windows (i = c + 8*a0), so the
    a2/a3 windows stop at 15. Total 66 super-blocks per core; the union of
    (owned I, I+w) over all cores covers every unordered block pair once.
    """
    out = []
    for a in range(NA):
        wmax = 16 if a < 2 else 15
        for w in range(wmax + 1):
            out.append((a, w))
    return out


def _upper_order():
    """Program-order list of the 62 strictly-upper (a, w) super-blocks."""
    return [(a, w) for a, w in _windows() if w > 0]


N_UP = len(_upper_order())  # 62


def build_nc():
    """Build the per-core SPMD Bass program.

    Inputs :  xt     [D, B]  fp8e4m3 — normalized, scaled x.T, columns
              rotated by 512*c so owned super-rows sit at local 8a.
              negeye [P, P]  f32 — the constant -2*scale^2 * I
    Outputs:  rowmax [P, 16]       f32 — [p, 4a+r] = row-max over the
              computed window for local row 4096a + 128r + p
              colmax [N_UP, 2048]  f32 — per strictly-upper super-block
              (program order), the 128-partition max of each of the
              4x512 (chunk, col) elements; host maxes over the 4 chunks.
    """
    f32 = mybir.dt.float32
    fp8 = mybir.dt.float8e4
    ngrp = KCH // KSTEP

    nc = bacc.Bacc("TRN2", target_bir_lowering=False, debug=False,
                   num_devices=N_CORES)
    xt = nc.dram_tensor("xt", [D, B], fp8, kind="ExternalInput")
    negeye = nc.dram_tensor("negeye", [P, P], f32, kind="ExternalInput")
    rowmax_d = nc.dram_tensor("rowmax", [P, NA * 4], f32,
                              kind="ExternalOutput")
    colmax_d = nc.dram_tensor("colmax", [N_UP, 4 * NBLK], f32,
                              kind="ExternalOutput")
    xt_ap = xt[:]
    colmax_ap = colmax_d[:]

    with tile.TileContext(nc) as tc:
        with (
            tc.tile_pool(name="lhs", bufs=1) as lhs_pool,
            tc.tile_pool(name="rhs", bufs=3) as rhs_pool,
            tc.tile_pool(name="psum", bufs=4, space="PSUM") as psum_pool,
            tc.tile_pool(name="stage", bufs=2) as stage_pool,
            tc.tile_pool(name="pred", bufs=2) as pred_pool,
            tc.tile_pool(name="stats", bufs=1) as stats_pool,
        ):
            # HWDGE queues on the SP and ACT engines; Pool is busy with the
            # partition reduces and SWDGE DMA issue would eat Pool time.
            dma_eng = [nc.sync, nc.scalar]
            ndma = 0

            lhs_tiles = []
            for a in range(NA):
                t = lhs_pool.tile([P, KCH, NBLK], fp8, name=f"lhs{a}",
                                  tag=f"lhs{a}")
                lhs_tiles.append(t)
                base = a * 8 * NBLK
                for k in range(KCH):
                    dma_eng[ndma % 2].dma_start(
                        t[:, k, :], xt_ap[k * P:(k + 1) * P, base:base + NBLK])
                    ndma += 1
            eye = stats_pool.tile([P, P], f32, name="eye")
            nc.sync.dma_start(eye[:], negeye[:])

            maxt = [
                stats_pool.tile([P, 4, 17], f32, name=f"maxt{a}",
                                tag=f"maxt{a}")
                for a in range(NA)
            ]
            rowmax_sb = stats_pool.tile([P, NA * 4], f32, name="rowmax_sb")

            sb_idx = 0  # strictly-upper super-block output row
            for a, w in _windows():
                J = (8 * a + w) % NSB
                if w == 0:
                    rt = None  # rhs block is the resident lhs tile itself
                else:
                    rt = rhs_pool.tile([P, KCH, NBLK], fp8, name="rt",
                                       tag="rt")
                    for k in range(KCH):
                        dma_eng[ndma % 2].dma_start(
                            rt[:, k, :],
                            xt_ap[k * P:(k + 1) * P,
                                  J * NBLK:(J + 1) * NBLK])
                        ndma += 1

                # Two 2-bank psum tiles per super-block (chunks r = 2h+rr).
                ps_tiles = []
                for h in range(2):
                    ps = psum_pool.tile([P, 2, NBLK], f32, name="ps",
                                        tag="ps")
                    ps_tiles.append(ps)
                    for rr in range(2):
                        r = 2 * h + rr
                        for g in range(ngrp):
                            ks = slice(KSTEP * g, KSTEP * (g + 1))
                            rhs = (lhs_tiles[a][:, ks, :] if rt is None
                                   else rt[:, ks, :])
                            nc.tensor.matmul(
                                ps[:, rr, :],
                                lhs_tiles[a][:, ks, r * P:(r + 1) * P],
                                rhs,
                                start=(g == 0),
                                stop=(g == ngrp - 1),
                                perf_mode=mybir.MatmulPerfMode.DoubleRow,
                            )

                if w == 0:
                    # self-similarity of chunk r lives at [p, r*P + p]:
                    # add -2*scale^2*I so the self-match never wins.
                    for r in range(4):
                        h, rr = divmod(r, 2)
                        sl = ps_tiles[h][:, rr, r * P:(r + 1) * P]
                        nc.vector.tensor_add(out=sl, in0=sl, in1=eye[:])

                for h in range(2):
                    nc.vector.reduce_max(
                        out=maxt[a][:, 2 * h:2 * h + 2, w:w + 1],
                        in_=ps_tiles[h][:],
                        axis=mybir.AxisListType.X,
                        op=mybir.AluOpType.max,
                    )

                if w > 0:
                    st = stage_pool.tile([P, 4, NBLK], f32, name="st",
                                         tag="st")
                    for r in range(4):
                        h, rr = divmod(r, 2)
                        nc.scalar.copy(st[:, r, :], ps_tiles[h][:, rr, :])
                    pr = pred_pool.tile([P, 4 * NBLK], f32, name="pr",
                                        tag="pr")
                    nc.gpsimd.partition_all_reduce(
                        pr[:], st[:], channels=P,
                        reduce_op=bass_isa.ReduceOp.max,
                    )
                    dma_eng[ndma % 2].dma_start(
                        colmax_ap[sb_idx:sb_idx + 1, :], pr[0:1, :])
                    ndma += 1
                    sb_idx += 1

            for a in range(NA):
                nw = 17 if a < 2 else 16
                nc.vector.reduce_max(
                    out=rowmax_sb[:, 4 * a:4 * a + 4],
                    in_=maxt[a][:, :, 0:nw],
                    axis=mybir.AxisListType.X,
                    op=mybir.AluOpType.max,
                )
            nc.sync.dma_start(rowmax_d[:], rowmax_sb[:])

    nc.compile()
    return nc


def prepare_inputs(x):
    """Host prep: normalize (f64), transpose, scale+cast fp8, rotate."""
    xd = np.asarray(x, dtype=np.float64)
    norms = np.sqrt(np.einsum("ij,ij->i", xd, xd))
    np.maximum(norms, 1e-12, out=norms)
    xn = xd / norms[:, None]
    xnt = np.ascontiguousarray(xn.T * FP8_SCALE).astype(ml_dtypes.float8_e4m3)
    negeye = np.ascontiguousarray(
        (-2.0 * FP8_SCALE * FP8_SCALE) * np.eye(P, dtype=np.float32))
    in_maps = []
    for c in range(N_CORES):
        s = c * NBLK
        rot = np.concatenate([xnt[:, s:], xnt[:, :s]], axis=1) if s else xnt
        in_maps.append({"xt": np.ascontiguousarray(rot), "negeye": negeye})
    return in_maps


def postprocess(results):
    """Merge per-core row/column maxima and apply the scalar epilogue."""
    inv = 1.0 / (FP8_SCALE * FP8_SCALE)
    order = _upper_order()
    maxsim = np.full(B, -np.inf, dtype=np.float64)
    for c in range(N_CORES):
        rm = np.asarray(results[c]["rowmax"], dtype=np.float64)  # [P, 16]
        for a in range(NA):
            for r in range(4):
                g0 = (c + 8 * a) * NBLK + r * P  # global row of partition 0
                sl = slice(g0, g0 + P)
                np.maximum(maxsim[sl], rm[:, 4 * a + r], out=maxsim[sl])
        cmx = np.asarray(results[c]["colmax"], dtype=np.float64)
        cmx = cmx.reshape(N_UP, 4, NBLK).max(axis=1)  # [N_UP, NBLK]
        for s, (a, w) in enumerate(order):
            g0 = ((8 * a + w + c) % NSB) * NBLK
            sl = slice(g0, g0 + NBLK)
            np.maximum(maxsim[sl], cmx[s], out=maxsim[sl])
    d2 = 2.0 - 2.0 * (maxsim * inv) + EPS
    loss = -0.5 * np.mean(np.log(d2))
    return np.array(loss, dtype=np.float32)


_NC_CACHE = {}


def _get_nc():
    if "nc" not in _NC_CACHE:
        _NC_CACHE["nc"] = build_nc()
    return _NC_CACHE["nc"]


def kernel(x, **_ignored):
    nc = _get_nc()
    in_maps = prepare_inputs(x)
    last_exc = None
    for _attempt in range(3):
        try:
            res = run_bass_kernel_spmd(nc, in_maps,
                                       core_ids=list(range(N_CORES)))
            return postprocess(res.results)
        except Exception as exc:  # transient NRT/tunnel hiccups
            last_exc = exc
    raise last_exc


if __name__ == "__main__":
    x = np.random.default_rng(0).standard_normal((B, D), dtype=np.float32)
    print(kernel(x))


# revision 12
# speedup vs baseline: 1.4810x; 1.4810x over previous
"""KoLeo loss kernel for 8 Trainium2 NeuronCores — symmetric (half-matrix)
variant.

Reference computation (B=16384, D=1024):
    xn  = x / max(||x||_2, 1e-12)          # row L2-normalize
    sim = xn @ xn.T                        # B x B cosine similarity
    max_sim[i] = max_{j != i} sim[i, j]    # nearest neighbor (excl. self)
    out = -mean(log(sqrt(2 - 2*max_sim + 1e-8)))

sim is symmetric, so only the upper triangle of 512x512 super-blocks is
computed (~53% of the matmul work of the dense baseline). Each computed
super-block (I, J), I < J, serves rows of I via its row-max and rows of J
via its column-max:
  - row-max: DVE free-axis reduce straight from PSUM (one [128,4,512] op).
  - column-max: ACT copies the whole 4-bank PSUM tile to fp16 SBUF in one
    op (Pool cannot touch PSUM on TRN2, and ACT per-instruction overhead
    makes small copies expensive), DVE folds the 4 chunk rows into one
    [128,512] fp16 tile (2 tensor_tensor maxes in the 16-bit 2x mode),
    Pool partition_all_reduces the folded tile (max over 128 partitions,
    measured ~3.4 ns per output element on the Q7s, so only the 512-wide
    folded tile is affordable), and one replicated partition row [1, 512]
    is DMA'd out per super-block.
This keeps the Tensor engine the critical path: per core 66 super-blocks
= 1056 fp8 DoubleRow matmuls (~229 us at the 157 TF/s fp8 roofline, vs
437 us for the dense baseline), with DVE ~220 us, ACT ~150 us and Pool
~125 us underneath.

Work distribution ("pencil window", SPMD-uniform): global super-row G is
owned by core c = G % 8. Each core holds its 4 owned super-rows resident
(local positions 8a after a per-core rotation of x.T columns by 512*c) and
computes super-blocks (I, I+w mod 32) for w = 0..16 (a in {0,1}) or
w = 0..15 (a in {2,3}). Every unordered pair of super-blocks is covered
exactly once across the fleet (528 total); w=0 is the diagonal block,
where a -2*scale^2*I constant is added at the self-similarity positions
before the row max so the self-match never wins.

Host: pre-normalizes rows (f64), scales by 8 and casts to fp8e4m3,
pre-rotates columns per core; post-merges row/column maxima across cores
and applies the scalar log epilogue in f64.
"""

import sys

if "/opt/trn_rl_repo" not in sys.path:
    sys.path.insert(0, "/opt/trn_rl_repo")

import numpy as np
import ml_dtypes

import concourse.bass as bass  # noqa: F401  (import keeps bass registered)
import concourse.bass_isa as bass_isa
import concourse.mybir as mybir
import concourse.tile as tile
from concourse import bacc
from concourse.bass_utils import run_bass_kernel_spmd

P = 128          # SBUF partitions
NBLK = 512       # super-block side (= one PSUM bank of f32 per 128 rows)
EPS = 1e-8

B = 16384        # rows of x
D = 1024         # feature dim
N_CORES = 8
NSB = B // NBLK  # 32 super-blocks per matrix side
KCH = D // P     # 8 contraction chunks of 128
KSTEP = 2        # fp8 DoubleRow: K chunks of 256 per matmul
NA = 4           # owned super-rows per core (global stride 8)
FP8_SCALE = 8.0


def _windows():
    """Program-order (a, w) list. w=0 is the diagonal super-block.

    a in {0,1} get w up to 16, a in {2,3} up to 15: the distance-16 pairs
    {i, i+16} are covered once by the a0/a1 windows (i = c + 8*a0), so the
    a2/a3 windows stop at 15. Total 66 super-blocks per core; the union of
    (owned I, I+w) over all cores covers every unordered block pair once.
    """
    out = []
    for a in range(NA):
        wmax = 16 if a < 2 else 15
        for w in range(wmax + 1):
            out.append((a, w))
    return out


def _upper_order():
    """Program-order list of the 62 strictly-upper (a, w) super-blocks."""
    return [(a, w) for a, w in _windows() if w > 0]


N_UP = len(_upper_order())  # 62


def build_nc():
    """Build the per-core SPMD Bass program.

    Inputs :  xt     [D, B]  fp8e4m3 — normalized, scaled x.T, columns
              rotated by 512*c so owned super-rows sit at local 8a.
              negeye [P, P]  f32 — the constant -2*scale^2 * I
    Outputs:  rowmax [P, 16]       f32 — [p, 4a+r] = row-max over the
              computed window for local row 4096a + 128r + p
              colmax [N_UP, NBLK]  f16 — per strictly-upper super-block
              (program order), the 512-row column max of the block.
    """
    f32 = mybir.dt.float32
    f16 = mybir.dt.float16
    fp8 = mybir.dt.float8e4
    ngrp = KCH // KSTEP

    nc = bacc.Bacc("TRN2", target_bir_lowering=False, debug=False,
                   num_devices=N_CORES)
    xt = nc.dram_tensor("xt", [D, B], fp8, kind="ExternalInput")
    negeye = nc.dram_tensor("negeye", [P, P], f32, kind="ExternalInput")
    rowmax_d = nc.dram_tensor("rowmax", [P, NA * 4], f32,
                              kind="ExternalOutput")
    colmax_d = nc.dram_tensor("colmax", [N_UP, NBLK], f16,
                              kind="ExternalOutput")
    xt_ap = xt[:]
    colmax_ap = colmax_d[:]

    with tile.TileContext(nc) as tc:
        with (
            tc.tile_pool(name="lhs", bufs=1) as lhs_pool,
            tc.tile_pool(name="rhs", bufs=3) as rhs_pool,
            tc.tile_pool(name="psum", bufs=2, space="PSUM") as psum_pool,
            tc.tile_pool(name="stage", bufs=2) as stage_pool,
            tc.tile_pool(name="pred", bufs=2) as pred_pool,
            tc.tile_pool(name="stats", bufs=1) as stats_pool,
        ):
            # HWDGE queues on the SP and ACT engines; Pool is busy with the
            # partition reduces and SWDGE DMA issue would eat Pool time.
            dma_eng = [nc.sync, nc.scalar]
            ndma = 0

            lhs_tiles = []
            for a in range(NA):
                t = lhs_pool.tile([P, KCH, NBLK], fp8, name=f"lhs{a}",
                                  tag=f"lhs{a}")
                lhs_tiles.append(t)
                base = a * 8 * NBLK
                for k in range(KCH):
                    dma_eng[ndma % 2].dma_start(
                        t[:, k, :], xt_ap[k * P:(k + 1) * P, base:base + NBLK])
                    ndma += 1
            eye = stats_pool.tile([P, P], f32, name="eye")
            nc.sync.dma_start(eye[:], negeye[:])

            maxt = [
                stats_pool.tile([P, 4, 17], f32, name=f"maxt{a}",
                                tag=f"maxt{a}")
                for a in range(NA)
            ]
            rowmax_sb = stats_pool.tile([P, NA * 4], f32, name="rowmax_sb")

            sb_idx = 0  # strictly-upper super-block output row
            for a, w in _windows():
                J = (8 * a + w) % NSB
                if w == 0:
                    rt = None  # rhs block is the resident lhs tile itself
                else:
                    rt = rhs_pool.tile([P, KCH, NBLK], fp8, name="rt",
                                       tag="rt")
                    for k in range(KCH):
                        dma_eng[ndma % 2].dma_start(
                            rt[:, k, :],
                            xt_ap[k * P:(k + 1) * P,
                                  J * NBLK:(J + 1) * NBLK])
                        ndma += 1

                # One 4-bank psum tile per super-block (chunks r = 0..3).
                ps = psum_pool.tile([P, 4, NBLK], f32, name="ps", tag="ps")
                for r in range(4):
                    for g in range(ngrp):
                        ks = slice(KSTEP * g, KSTEP * (g + 1))
                        rhs = (lhs_tiles[a][:, ks, :] if rt is None
                               else rt[:, ks, :])
                        nc.tensor.matmul(
                            ps[:, r, :],
                            lhs_tiles[a][:, ks, r * P:(r + 1) * P],
                            rhs,
                            start=(g == 0),
                            stop=(g == ngrp - 1),
                            perf_mode=mybir.MatmulPerfMode.DoubleRow,
                        )

                if w == 0:
                    # self-similarity of chunk r lives at [p, r*P + p]:
                    # add -2*scale^2*I so the self-match never wins.
                    for r in range(4):
                        sl = ps[:, r, r * P:(r + 1) * P]
                        nc.vector.tensor_add(out=sl, in0=sl, in1=eye[:])

                nc.vector.reduce_max(
                    out=maxt[a][:, :, w:w + 1],
                    in_=ps[:],
                    axis=mybir.AxisListType.X,
                    op=mybir.AluOpType.max,
                )

                if w > 0:
                    st = stage_pool.tile([P, 4, NBLK], f16, name="st",
                                         tag="st")
                    nc.scalar.copy(st[:], ps[:])
                    stm = stage_pool.tile([P, 2, NBLK], f16, name="stm",
                                          tag="stm")
                    nc.vector.tensor_max(out=stm[:], in0=st[:, 0:2, :],
                                         in1=st[:, 2:4, :])
                    mg = stage_pool.tile([P, NBLK], f16, name="mg", tag="mg")
                    nc.vector.tensor_max(out=mg[:], in0=stm[:, 0, :],
                                         in1=stm[:, 1, :])
                    pm = pred_pool.tile([P, NBLK], f16, name="pm", tag="pm")
                    nc.gpsimd.partition_all_reduce(
                        pm[:], mg[:], channels=P,
                        reduce_op=bass_isa.ReduceOp.max,
                    )
                    dma_eng[ndma % 2].dma_start(
                        colmax_ap[sb_idx:sb_idx + 1, :], pm[0:1, :])
                    ndma += 1
                    sb_idx += 1

            for a in range(NA):
                nw = 17 if a < 2 else 16
                nc.vector.reduce_max(
                    out=rowmax_sb[:, 4 * a:4 * a + 4],
                    in_=maxt[a][:, :, 0:nw],
                    axis=mybir.AxisListType.X,
                    op=mybir.AluOpType.max,
                )
            nc.sync.dma_start(rowmax_d[:], rowmax_sb[:])

    nc.compile()
    return nc


def prepare_inputs(x):
    """Host prep: normalize (f64), transpose, scale+cast fp8, rotate."""
    xd = np.asarray(x, dtype=np.float64)
    norms = np.sqrt(np.einsum("ij,ij->i", xd, xd))
    np.maximum(norms, 1e-12, out=norms)
    xn = xd / norms[:, None]
    xnt = np.ascontiguousarray(xn.T * FP8_SCALE).astype(ml_dtypes.float8_e4m3)
    negeye = np.ascontiguousarray(
        (-2.0 * FP8_SCALE * FP8_SCALE) * np.eye(P, dtype=np.float32))
    in_maps = []
    for c in range(N_CORES):
        s = c * NBLK
        rot = np.concatenate([xnt[:, s:], xnt[:, :s]], axis=1) if s else xnt
        in_maps.append({"xt": np.ascontiguousarray(rot), "negeye": negeye})
    return in_maps


def postprocess(results):
    """Merge per-core row/column maxima and apply the scalar epilogue."""
    inv = 1.0 / (FP8_SCALE * FP8_SCALE)
    order = _upper_order()
    maxsim = np.full(B, -np.inf, dtype=np.float64)
    for c in range(N_CORES):
        rm = np.asarray(results[c]["rowmax"], dtype=np.float64)  # [P, 16]
        for a in range(NA):
            for r in range(4):
                g0 = (c + 8 * a) * NBLK + r * P  # global row of partition 0
                sl = slice(g0, g0 + P)
                np.maximum(maxsim[sl], rm[:, 4 * a + r], out=maxsim[sl])
        cmx = np.asarray(results[c]["colmax"], dtype=np.float64)  # [N_UP, NBLK]
        for s, (a, w) in enumerate(order):
            g0 = ((8 * a + w + c) % NSB) * NBLK
            sl = slice(g0, g0 + NBLK)
            np.maximum(maxsim[sl], cmx[s], out=maxsim[sl])
    d2 = 2.0 - 2.0 * (maxsim * inv) + EPS
    loss = -0.5 * np.mean(np.log(d2))
    return np.array(loss, dtype=np.float32)


_NC_CACHE = {}


def _get_nc():
    if "nc" not in _NC_CACHE:
        _NC_CACHE["nc"] = build_nc()
    return _NC_CACHE["nc"]


def kernel(x, **_ignored):
    nc = _get_nc()
    in_maps = prepare_inputs(x)
    last_exc = None
    for _attempt in range(3):
        try:
            res = run_bass_kernel_spmd(nc, in_maps,
                                       core_ids=list(range(N_CORES)))
            return postprocess(res.results)
        except Exception as exc:  # transient NRT/tunnel hiccups
            last_exc = exc
    raise last_exc


if __name__ == "__main__":
    x = np.random.default_rng(0).standard_normal((B, D), dtype=np.float32)
    print(kernel(x))
